# revision 3
# baseline (speedup 1.0000x reference)
"""MoE layer (top-2 routing, 8 experts) on 8 Trainium2 NeuronCores.

Strategy — expert-parallel with hidden-dim (H) slicing for perfect balance:
  - Host computes the gate (router math in fp64 numpy): logits, top-2 experts
    per token, softmax gates; tokens are sorted into per-expert segments.
  - ReLU is elementwise in H, so each expert MLP decomposes exactly into 8
    independent H-slice MLPs (D x 512 x D). Core c holds slice c of EVERY
    expert (same 16.8MB fp16 weight footprint as one whole expert).
  - The kernel runs 8 passes; pass e = all 8 cores compute expert e's slice
    over exactly n_e tokens (identical shapes on every core -> SPMD, zero
    padding, perfect load balance).
  - Each core emits gate-weighted partial outputs; host sums the 8 cores'
    partials and scatter-adds each token's two expert contributions.

Schedule notes (from profile analysis): the PE stream is gap-free mid-run as
long as the x-tile prefetch queue (sync) carries nothing that is anchored to
the compute timeline, so y-output DMAs stay off it. Startup: x tiles (sync),
w1 q0 split across scalar+gpsimd, and b1/w2 packs 0-1 (gpsimd) all stream in
parallel, with pass-0 tiles ramped 128/256/384 so real matmuls start ~9us.
Later weight groups are emitted inside the pass loop at their consumption
points, which both paces them and keeps y DMAs from queueing behind them.
Tail: the last two tiles' y DMAs ride sync/scalar (gpsimd's slow queue-drain
then completes long before the final barrier) and the last tile is 128
tokens so the post-matmul chain is short.

Hardcoded problem shape: x(8192,1024) w1(8,1024,4096) w2(8,4096,1024).
"""

import numpy as np

import concourse.tile as tile
import concourse.mybir as mybir
from concourse import bacc
from concourse.bass_utils import run_bass_kernel_spmd

E = 8          # experts
D = 1024       # model dim
H = 4096       # hidden dim
HS = H // 8    # per-core hidden slice (512)
NHS = HS // 128  # h-tiles per slice (4)
TOP_K = 2
N_CORES = 8
ND = D // 128   # 8 d-tiles

F32 = mybir.dt.float32
F16 = mybir.dt.float16


def _balanced(n, max_tile=512):
    """Near-equal split of n into ceil(n/max_tile) tiles."""
    nt = max(1, -(-n // max_tile))
    base, rem = divmod(n, nt)
    return [base + (1 if i < rem else 0) for i in range(nt)]


def _pass_sizes(n, first=False, last=False):
    if first and n >= 1280:
        return [128, 256, 384] + _balanced(n - 768)
    if last and n >= 768:
        return _balanced(n - 128) + [128]
    return _balanced(n)


def build_moe(counts):
    """Build + compile the 8-pass H-sliced expert MLP program.

    counts: per-expert token counts (same on every core; pass e covers
    exactly counts[e] tokens). Weight/x/g/y DRAM tensors hold the per-core
    slice data laid out expert-major (see moe_run for host layouts).
    """
    total = int(sum(counts))
    starts = np.concatenate([[0], np.cumsum(counts)]).astype(int)

    nc = bacc.Bacc("TRN2", target_bir_lowering=False, debug=False, num_devices=N_CORES)

    xt = nc.dram_tensor("xt", [D, total], F16, kind="ExternalInput")   # sorted x^T
    w1 = nc.dram_tensor("w1", [D, E * HS], F16, kind="ExternalInput")  # cols e*512..: this core's slice of expert e
    w2 = nc.dram_tensor("w2", [E * HS, D], F16, kind="ExternalInput")  # rows e*512..: this core's slice of expert e
    b1 = nc.dram_tensor("b1", [128, E * NHS], F32, kind="ExternalInput")
    g = nc.dram_tensor("g", [128, total], F16, kind="ExternalInput")   # gates, replicated rows
    yt = nc.dram_tensor("yt", [D, total], F16, kind="ExternalOutput")

    xt_ap, w1_ap, w2_ap, b1_ap, g_ap, yt_ap = (
        t.ap() for t in (xt, w1, w2, b1, g, yt)
    )

    pass_tiles = []
    for e in range(E):
        szs = _pass_sizes(int(counts[e]), first=(e == 0), last=(e == E - 1))
        t0 = int(starts[e])
        tl = []
        for s in szs:
            tl.append((t0, s))
            t0 += s
        pass_tiles.append(tl)
    n_tiles_total = sum(len(t) for t in pass_tiles)

    with tile.TileContext(nc) as tc:
        with (
            tc.tile_pool(name="wpool", bufs=1) as wpool,
            tc.tile_pool(name="xpool", bufs=3) as xpool,
            tc.tile_pool(name="hpool", bufs=10) as hpool,
            tc.tile_pool(name="ypool", bufs=6) as ypool,
            tc.tile_pool(name="gpool", bufs=4) as gpool,
            tc.tile_pool(name="ph", bufs=4, space="PSUM") as ph_pool,
            tc.tile_pool(name="py", bufs=4, space="PSUM") as py_pool,
        ):
            def load_gate(t0, tn):
                g_sb = gpool.tile([128, 512], F16, name=f"gsb{t0}", tag="gsb")
                nc.sync.dma_start(g_sb[:, :tn], g_ap[:, t0:t0 + tn])
                return g_sb

            def load_tok_tile(t0, tn, granular=False):
                # d-slice j lives at columns [j*tn, (j+1)*tn) of a wide tile.
                xtile = xpool.tile([128, ND * 512], F16, name=f"xsb{t0}", tag="xsb")
                if granular:
                    # Paired per-d DMAs so the first d-chunks land (and the
                    # first matmuls fire) before the whole tile transfers.
                    for q in range(ND // 2):
                        src = xt_ap[q * 256:(q + 1) * 256, t0:t0 + tn].rearrange(
                            "(dd p) t -> p dd t", p=128)
                        dst = xtile[:, 2 * q * tn:(2 * q + 2) * tn].rearrange(
                            "p (dd t) -> p dd t", t=tn)
                        nc.sync.dma_start(dst, src)
                else:
                    src = xt_ap[:, t0:t0 + tn].rearrange("(dd p) t -> p dd t", p=128)
                    dst = xtile[:, :ND * tn].rearrange("p (dd t) -> p dd t", t=tn)
                    nc.sync.dma_start(dst, src)
                return [xtile[:, d * tn:(d + 1) * tn] for d in range(ND)]

            # Startup prefetch, three queues in parallel:
            #   sync:   x tile0 (granular) + gate0, x tile1 + gate1
            #   scalar: w1 q0 chunks d0..d3
            #   gpsimd: b1, w1 q0 chunks d4..d7, w2 packs 0 and 1
            prefetched = {pass_tiles[0][0][0]: load_tok_tile(*pass_tiles[0][0], granular=True)}
            g_prefetched = {pass_tiles[0][0][0]: load_gate(*pass_tiles[0][0])}
            t1 = pass_tiles[0][1][0]
            prefetched[t1] = load_tok_tile(*pass_tiles[0][1])
            g_prefetched[t1] = load_gate(*pass_tiles[0][1])
            b1_sb = wpool.tile([128, E * NHS], F32, name="b1sb", tag="b1sb")
            nc.gpsimd.dma_start(b1_sb[:], b1_ap[:, :])

            w1_sb = [[None] * E for _ in range(ND)]  # [d][e] -> [128, HS]

            def load_w1_group(q, engs):
                # [128, 1024] chunk per d: experts {2q, 2q+1}, 2KB DMA lines.
                for d in range(ND):
                    t = wpool.tile([128, 2 * HS], F16, name=f"w1c{d}_{q}", tag=f"w1c{d}_{q}")
                    engs[d % len(engs)].dma_start(
                        t[:], w1_ap[d * 128:(d + 1) * 128, q * 2 * HS:(q + 1) * 2 * HS]
                    )
                    w1_sb[d][2 * q] = t[:, :HS]
                    w1_sb[d][2 * q + 1] = t[:, HS:]

            w2_sb = [None] * E

            def load_w2(e, eng):
                t = wpool.tile([128, NHS * D], F16, name=f"w2p{e}", tag=f"w2p{e}")
                src = w2_ap[e * HS:(e + 1) * HS, :].rearrange("(ho p) d -> p ho d", p=128)
                dst = t.rearrange("p (ho d) -> p ho d", d=D)
                eng.dma_start(dst, src)
                w2_sb[e] = t

            # q0 feeds passes 0/1; split across two queues so the first
            # layer-1 chains are paced at ~2x single-queue DMA rate.
            load_w1_group(0, [nc.scalar, nc.scalar, nc.scalar, nc.scalar,
                              nc.gpsimd, nc.gpsimd, nc.gpsimd, nc.gpsimd][:ND])
            load_w2(0, nc.gpsimd)
            load_w2(1, nc.gpsimd)

            n_y = 0
            tile_idx = 0

            for e in range(E):
                for ti, (t0, tn) in enumerate(pass_tiles[e]):
                    x_sb = prefetched.pop(t0) if t0 in prefetched else load_tok_tile(t0, tn)
                    g_sb = g_prefetched.pop(t0) if t0 in g_prefetched else load_gate(t0, tn)

                    # Layer 1: H-slice^T[j] = relu(sum_d W1s[d, j]^T X^T[d] + b1s[j])
                    h_sb = []
                    for j in range(NHS):
                        ph = ph_pool.tile([128, 512], F32, name=f"ph{e}_{t0}_{j}", tag="ph")
                        for d in range(ND):
                            nc.tensor.matmul(
                                ph[:, :tn],
                                w1_sb[d][e][:, j * 128:(j + 1) * 128],
                                x_sb[d][:, :tn],
                                start=(d == 0),
                                stop=(d == ND - 1),
                            )
                        ht = hpool.tile([128, 512], F16, name=f"hsb{e}_{t0}_{j}", tag="hsb")
                        nc.vector.tensor_scalar(
                            ht[:, :tn], ph[:, :tn],
                            b1_sb[:, e * NHS + j:e * NHS + j + 1], 0.0,
                            op0=mybir.AluOpType.add, op1=mybir.AluOpType.max,
                        )
                        h_sb.append(ht)
                    if ti == 0:
                        # Later weight groups are emitted here, at the pass
                        # that precedes their first use by one (w1 group q
                        # feeds passes 2q/2q+1; w2 pack e feeds pass e).
                        # Queue FIFO order paces them behind earlier loads.
                        if e == 0:
                            load_w1_group(1, [nc.scalar])
                        elif e == 1:
                            load_w2(2, nc.scalar)
                        elif e == 2:
                            load_w1_group(2, [nc.scalar])
                            load_w2(3, nc.scalar)
                        elif e == 3:
                            load_w2(4, nc.scalar)
                        elif e == 4:
                            load_w1_group(3, [nc.scalar])
                            load_w2(5, nc.scalar)
                        elif e == 5:
                            load_w2(6, nc.scalar)
                        elif e == 6:
                            load_w2(7, nc.scalar)

                    # Layer 2: Y^T[do] += g * sum_j W2s[j, do]^T Hs^T[j]
                    # y DMAs: scalar/gpsimd mid-run; the last two tiles ride
                    # scalar then sync so every queue's drain starts early.
                    if tile_idx == n_tiles_total - 1:
                        ydma_engines = [nc.sync]
                    elif tile_idx == n_tiles_total - 2:
                        ydma_engines = [nc.scalar]
                    else:
                        ydma_engines = [nc.scalar, nc.gpsimd]
                    for do in range(ND):
                        py = py_pool.tile([128, 512], F32, name=f"py{e}_{t0}_{do}", tag="py")
                        for j in range(NHS):
                            nc.tensor.matmul(
                                py[:, :tn],
                                w2_sb[e][:, j * D + do * 128:j * D + (do + 1) * 128],
                                h_sb[j][:, :tn],
                                start=(j == 0),
                                stop=(j == NHS - 1),
                            )
                        y_sb = ypool.tile([128, 512], F16, name=f"ysb{e}_{t0}_{do}", tag="ysb")
                        nc.vector.tensor_mul(y_sb[:, :tn], py[:, :tn], g_sb[:, :tn])
                        eng = ydma_engines[n_y % len(ydma_engines)]
                        n_y += 1
                        eng.dma_start(yt_ap[do * 128:(do + 1) * 128, t0:t0 + tn], y_sb[:, :tn])
                    tile_idx += 1

    nc.compile()
    return nc


def _route(x, wg, bg):
    """Host router in fp64: per-token top-2 experts and softmax gates."""
    logits = x.astype(np.float64) @ wg.astype(np.float64).T + bg.astype(np.float64)
    top2 = np.argpartition(-logits, 1, axis=1)[:, :TOP_K]  # two largest, unordered
    vals = np.take_along_axis(logits, top2, axis=1)
    ex = np.exp(vals - vals.max(axis=1, keepdims=True))
    gates = ex / ex.sum(axis=1, keepdims=True)
    idxs, gs = [], []
    for e in range(E):
        mask = top2 == e
        rows = np.nonzero(mask.any(axis=1))[0]
        idxs.append(rows)
        gs.append(gates[mask].astype(np.float32))
    return idxs, gs


def moe_run(x, wg, bg, w1, b1, w2, b2, trace=False, trace_kwargs=None):
    x = np.ascontiguousarray(np.asarray(x, np.float32))
    wg = np.asarray(wg, np.float32)
    bg = np.asarray(bg, np.float32)
    w1 = np.asarray(w1, np.float32)
    b1 = np.asarray(b1, np.float32)
    w2 = np.asarray(w2, np.float32)
    b2 = np.asarray(b2, np.float32)
    B = x.shape[0]

    idxs, gs = _route(x, wg, bg)
    counts = [len(r) for r in idxs]
    total = sum(counts)

    nc = build_moe(counts)

    # Shared (identical on every core): sorted activations and gates.
    order = np.concatenate(idxs)
    xt_all = np.ascontiguousarray(x[order].T).astype(np.float16)       # (D, total)
    g_all = np.concatenate(gs).astype(np.float16)                      # (total,)
    g_rep = np.ascontiguousarray(np.broadcast_to(g_all, (128, total)))

    in_maps = []
    for c in range(N_CORES):
        # Core c's H-slice [c*512, (c+1)*512) of every expert.
        w1c = np.concatenate([w1[e][:, c * HS:(c + 1) * HS] for e in range(E)], axis=1)
        w2c = np.concatenate([w2[e][c * HS:(c + 1) * HS, :] for e in range(E)], axis=0)
        b1c = np.concatenate([b1[e][c * HS:(c + 1) * HS].reshape(NHS, 128).T
                              for e in range(E)], axis=1)
        in_maps.append({
            "xt": xt_all,
            "w1": w1c.astype(np.float16),
            "w2": w2c.astype(np.float16),
            "b1": np.ascontiguousarray(b1c),
            "g": g_rep,
        })

    kwargs = {}
    if trace:
        kwargs["trace"] = True
        if trace_kwargs:
            kwargs.update(trace_kwargs)
    res = run_bass_kernel_spmd(nc, in_maps, core_ids=list(range(N_CORES)), **kwargs)

    # Sum the 8 cores' H-slice partials, then scatter-add per-expert segments.
    ysum = res.results[0]["yt"].astype(np.float32)
    for c in range(1, N_CORES):
        ysum += res.results[c]["yt"].astype(np.float32)

    out = np.zeros((B, D), np.float32)
    t = 0
    for e in range(E):
        n = counts[e]
        out[idxs[e]] += ysum[:, t:t + n].T + gs[e][:, None] * b2[e][None, :]
        t += n
    return out, res


def kernel(x, wg, bg, w1, b1, w2, b2):
    out, _ = moe_run(x, wg, bg, w1, b1, w2, b2, trace=False)
    return out


# revision 4
# speedup vs baseline: 1.0543x; 1.0543x over previous
"""MoE layer (top-2 routing, 8 experts) on 8 Trainium2 NeuronCores.

Strategy — expert-parallel with hidden-dim (H) slicing for perfect balance:
  - Host computes the gate (router math in fp64 numpy): logits, top-2 experts
    per token, softmax gates; tokens are sorted into per-expert segments.
  - ReLU is elementwise in H, so each expert MLP decomposes exactly into 8
    independent H-slice MLPs (D x 512 x D). Core c holds slice c of EVERY
    expert (same 16.8MB fp16 weight footprint as one whole expert).
  - The kernel runs 8 passes; pass e = all 8 cores compute expert e's slice
    over exactly n_e tokens (identical shapes on every core -> SPMD, zero
    padding, perfect load balance).
  - Each core emits gate-weighted partial outputs; host sums the 8 cores'
    partials and scatter-adds each token's two expert contributions.

Schedule notes (from profile analysis). Measured queue rates: sync
~250-290GB/s, scalar ~100-130GB/s, gpsimd similar with a slow start — so
everything startup-critical rides sync, finely interleaved (x tile0 pairs
with expert-0's w1 chunks, so the first matmul fires ~9us and layer 1 of
tile0 is paced by just 1MB of weights). Pass-0 tiles are ramped 128/256/384:
their compute at mid-pstate (~42us) far exceeds the remaining startup DMA.
The whole run is software-pipelined one tile deep (L1 of tile k+1 issues
before L2 of tile k) so a late w2 pack never blocks the next layer-1.
Mid-run invariants: sync carries only x+g (y DMAs would anchor the x
prefetch to the compute timeline); y rides gpsimd alone (sharing a queue
with bursty weight loads delays y completions, which blocks vector on ypool
recycle and in turn stalls the PE on PSUM recycle); later weight groups are
emitted in-loop on scalar, which carries nothing else. Tail: the last two
tiles' y DMAs ride scalar then sync and the final tile is 128 tokens, so
gpsimd's slow queue-drain finishes long before the final barrier.

Hardcoded problem shape: x(8192,1024) w1(8,1024,4096) w2(8,4096,1024).
"""

import numpy as np

import concourse.tile as tile
import concourse.mybir as mybir
from concourse import bacc
from concourse.bass_utils import run_bass_kernel_spmd

E = 8          # experts
D = 1024       # model dim
H = 4096       # hidden dim
HS = H // 8    # per-core hidden slice (512)
NHS = HS // 128  # h-tiles per slice (4)
TOP_K = 2
N_CORES = 8
ND = D // 128   # 8 d-tiles

F32 = mybir.dt.float32
F16 = mybir.dt.float16


def _balanced(n, max_tile=512):
    """Near-equal split of n into ceil(n/max_tile) tiles."""
    nt = max(1, -(-n // max_tile))
    base, rem = divmod(n, nt)
    return [base + (1 if i < rem else 0) for i in range(nt)]


def _pass_sizes(n, first=False, last=False):
    if first and n >= 1280:
        return [128, 256, 384] + _balanced(n - 768)
    if last and n >= 768:
        return _balanced(n - 128) + [128]
    return _balanced(n)


def build_moe(counts):
    """Build + compile the 8-pass H-sliced expert MLP program.

    counts: per-expert token counts (same on every core; pass e covers
    exactly counts[e] tokens). Weight/x/g/y DRAM tensors hold the per-core
    slice data laid out expert-major (see moe_run for host layouts).
    """
    total = int(sum(counts))
    starts = np.concatenate([[0], np.cumsum(counts)]).astype(int)

    nc = bacc.Bacc("TRN2", target_bir_lowering=False, debug=False, num_devices=N_CORES)

    xt = nc.dram_tensor("xt", [D, total], F16, kind="ExternalInput")   # sorted x^T
    w1 = nc.dram_tensor("w1", [D, E * HS], F16, kind="ExternalInput")  # cols e*512..: this core's slice of expert e
    w2 = nc.dram_tensor("w2", [E * HS, D], F16, kind="ExternalInput")  # rows e*512..: this core's slice of expert e
    b1 = nc.dram_tensor("b1", [128, E * NHS], F32, kind="ExternalInput")
    g = nc.dram_tensor("g", [128, total], F16, kind="ExternalInput")   # gates, replicated rows
    yt = nc.dram_tensor("yt", [D, total], F16, kind="ExternalOutput")

    xt_ap, w1_ap, w2_ap, b1_ap, g_ap, yt_ap = (
        t.ap() for t in (xt, w1, w2, b1, g, yt)
    )

    # Flat tile list [(e, t0, tn)], ramped at the very start and tiny at the
    # very end.
    tiles = []
    for e in range(E):
        szs = _pass_sizes(int(counts[e]), first=(e == 0), last=(e == E - 1))
        t0 = int(starts[e])
        for s in szs:
            tiles.append((e, t0, s))
            t0 += s
    T = len(tiles)
    pass_first = {}
    for k, (e, t0, tn) in enumerate(tiles):
        pass_first.setdefault(e, k)

    with tile.TileContext(nc) as tc:
        with (
            tc.tile_pool(name="wpool", bufs=1) as wpool,
            tc.tile_pool(name="xpool", bufs=3) as xpool,
            tc.tile_pool(name="hpool", bufs=10) as hpool,
            tc.tile_pool(name="ypool", bufs=6) as ypool,
            tc.tile_pool(name="gpool", bufs=4) as gpool,
            tc.tile_pool(name="ph", bufs=4, space="PSUM") as ph_pool,
            tc.tile_pool(name="py", bufs=4, space="PSUM") as py_pool,
        ):
            def load_gate(t0, tn):
                g_sb = gpool.tile([128, 512], F16, name=f"gsb{t0}", tag="gsb")
                nc.sync.dma_start(g_sb[:, :tn], g_ap[:, t0:t0 + tn])
                return g_sb

            w1_sb = [[None] * E for _ in range(ND)]  # [d][e] -> [128, HS]

            def load_w1_e(e, d, eng):
                # Single-expert [128, 512] chunk (1KB DMA lines).
                t = wpool.tile([128, HS], F16, name=f"w1e{e}d{d}", tag=f"w1e{e}d{d}")
                eng.dma_start(t[:], w1_ap[d * 128:(d + 1) * 128, e * HS:(e + 1) * HS])
                w1_sb[d][e] = t[:]

            def load_w1_group(q, eng):
                # [128, 1024] chunk per d: experts {2q, 2q+1}, 2KB DMA lines.
                for d in range(ND):
                    t = wpool.tile([128, 2 * HS], F16, name=f"w1c{d}_{q}", tag=f"w1c{d}_{q}")
                    eng.dma_start(
                        t[:], w1_ap[d * 128:(d + 1) * 128, q * 2 * HS:(q + 1) * 2 * HS]
                    )
                    w1_sb[d][2 * q] = t[:, :HS]
                    w1_sb[d][2 * q + 1] = t[:, HS:]

            w2_sb = [None] * E

            def load_w2(e, eng):
                t = wpool.tile([128, NHS * D], F16, name=f"w2p{e}", tag=f"w2p{e}")
                src = w2_ap[e * HS:(e + 1) * HS, :].rearrange("(ho p) d -> p ho d", p=128)
                dst = t.rearrange("p (ho d) -> p ho d", d=D)
                eng.dma_start(dst, src)
                w2_sb[e] = t

            def load_tok_tile(t0, tn, granular=False):
                # d-slice j lives at columns [j*tn, (j+1)*tn) of a wide tile.
                xtile = xpool.tile([128, ND * 512], F16, name=f"xsb{t0}", tag="xsb")
                if granular:
                    # Paired per-d DMAs interleaved with w1 chunks so the
                    # first matmuls fire as soon as the first pieces land.
                    for q in range(ND // 2):
                        src = xt_ap[q * 256:(q + 1) * 256, t0:t0 + tn].rearrange(
                            "(dd p) t -> p dd t", p=128)
                        dst = xtile[:, 2 * q * tn:(2 * q + 2) * tn].rearrange(
                            "p (dd t) -> p dd t", t=tn)
                        nc.sync.dma_start(dst, src)
                        load_w1_e(0, 2 * q, nc.sync)
                        if q == 0:
                            load_w1_e(0, 1, nc.sync)
                        else:
                            load_w1_e(0, 2 * q + 1, nc.sync)
                else:
                    src = xt_ap[:, t0:t0 + tn].rearrange("(dd p) t -> p dd t", p=128)
                    dst = xtile[:, :ND * tn].rearrange("p (dd t) -> p dd t", t=tn)
                    nc.sync.dma_start(dst, src)
                return [xtile[:, d * tn:(d + 1) * tn] for d in range(ND)]

            # Startup, all on sync in consumption order:
            #   x0-pairs/w1-e0 interleaved, g0, b1, x1, g1, w2p0, x2, g2
            prefetched = {tiles[0][1]: load_tok_tile(tiles[0][1], tiles[0][2], granular=True)}
            g_prefetched = {tiles[0][1]: load_gate(tiles[0][1], tiles[0][2])}
            b1_sb = wpool.tile([128, E * NHS], F32, name="b1sb", tag="b1sb")
            nc.sync.dma_start(b1_sb[:], b1_ap[:, :])
            prefetched[tiles[1][1]] = load_tok_tile(tiles[1][1], tiles[1][2])
            g_prefetched[tiles[1][1]] = load_gate(tiles[1][1], tiles[1][2])
            load_w2(0, nc.sync)
            prefetched[tiles[2][1]] = load_tok_tile(tiles[2][1], tiles[2][2])
            g_prefetched[tiles[2][1]] = load_gate(tiles[2][1], tiles[2][2])

            # Expert 1's w1 half and all later weight groups ride scalar
            # (slow queue, nothing else on it — a burst never delays y).
            for d in range(ND):
                load_w1_e(1, d, nc.scalar)
            load_w2(1, nc.scalar)

            def emit_weight_loads(e):
                # At pass e's first tile, start the loads consumed one or two
                # passes later; scalar FIFO order paces them.
                if e == 0:
                    load_w1_group(1, nc.scalar)
                elif e == 1:
                    load_w2(2, nc.scalar)
                elif e == 2:
                    load_w1_group(2, nc.scalar)
                    load_w2(3, nc.scalar)
                elif e == 3:
                    load_w2(4, nc.scalar)
                elif e == 4:
                    load_w1_group(3, nc.scalar)
                    load_w2(5, nc.scalar)
                elif e == 5:
                    load_w2(6, nc.scalar)
                elif e == 6:
                    load_w2(7, nc.scalar)

            n_y = 0

            def emit_l2(k, h_sb, g_sb):
                e, t0, tn = tiles[k]
                # y DMAs: gpsimd mid-run; the last two tiles ride scalar then
                # sync so every queue's drain starts before the final barrier.
                if k == T - 1:
                    ydma_engines = [nc.sync]
                elif k == T - 2:
                    ydma_engines = [nc.scalar]
                else:
                    ydma_engines = [nc.gpsimd]
                nonlocal n_y
                for do in range(ND):
                    py = py_pool.tile([128, 512], F32, name=f"py{e}_{t0}_{do}", tag="py")
                    for j in range(NHS):
                        nc.tensor.matmul(
                            py[:, :tn],
                            w2_sb[e][:, j * D + do * 128:j * D + (do + 1) * 128],
                            h_sb[j][:, :tn],
                            start=(j == 0),
                            stop=(j == NHS - 1),
                        )
                    y_sb = ypool.tile([128, 512], F16, name=f"ysb{e}_{t0}_{do}", tag="ysb")
                    nc.vector.tensor_mul(y_sb[:, :tn], py[:, :tn], g_sb[:, :tn])
                    eng = ydma_engines[n_y % len(ydma_engines)]
                    n_y += 1
                    eng.dma_start(yt_ap[do * 128:(do + 1) * 128, t0:t0 + tn], y_sb[:, :tn])

            pending = None  # (k, h_sb, g_sb) awaiting layer 2

            for k, (e, t0, tn) in enumerate(tiles):
                x_sb = prefetched.pop(t0) if t0 in prefetched else load_tok_tile(t0, tn)
                g_sb = g_prefetched.pop(t0) if t0 in g_prefetched else load_gate(t0, tn)

                # Layer 1: H-slice^T[j] = relu(sum_d W1s[d, j]^T X^T[d] + b1s[j])
                h_sb = []
                for j in range(NHS):
                    ph = ph_pool.tile([128, 512], F32, name=f"ph{e}_{t0}_{j}", tag="ph")
                    for d in range(ND):
                        nc.tensor.matmul(
                            ph[:, :tn],
                            w1_sb[d][e][:, j * 128:(j + 1) * 128],
                            x_sb[d][:, :tn],
                            start=(d == 0),
                            stop=(d == ND - 1),
                        )
                    ht = hpool.tile([128, 512], F16, name=f"hsb{e}_{t0}_{j}", tag="hsb")
                    nc.vector.tensor_scalar(
                        ht[:, :tn], ph[:, :tn],
                        b1_sb[:, e * NHS + j:e * NHS + j + 1], 0.0,
                        op0=mybir.AluOpType.add, op1=mybir.AluOpType.max,
                    )
                    h_sb.append(ht)
                if pass_first.get(e) == k:
                    emit_weight_loads(e)

                # One-tile software pipeline: layer 2 of tile k-1 issues here,
                # after layer 1 of tile k.
                if pending is not None:
                    emit_l2(*pending)
                pending = (k, h_sb, g_sb)

            emit_l2(*pending)

    nc.compile()
    return nc


def _route(x, wg, bg):
    """Host router in fp64: per-token top-2 experts and softmax gates."""
    logits = x.astype(np.float64) @ wg.astype(np.float64).T + bg.astype(np.float64)
    top2 = np.argpartition(-logits, 1, axis=1)[:, :TOP_K]  # two largest, unordered
    vals = np.take_along_axis(logits, top2, axis=1)
    ex = np.exp(vals - vals.max(axis=1, keepdims=True))
    gates = ex / ex.sum(axis=1, keepdims=True)
    idxs, gs = [], []
    for e in range(E):
        mask = top2 == e
        rows = np.nonzero(mask.any(axis=1))[0]
        idxs.append(rows)
        gs.append(gates[mask].astype(np.float32))
    return idxs, gs


def moe_run(x, wg, bg, w1, b1, w2, b2, trace=False, trace_kwargs=None):
    x = np.ascontiguousarray(np.asarray(x, np.float32))
    wg = np.asarray(wg, np.float32)
    bg = np.asarray(bg, np.float32)
    w1 = np.asarray(w1, np.float32)
    b1 = np.asarray(b1, np.float32)
    w2 = np.asarray(w2, np.float32)
    b2 = np.asarray(b2, np.float32)
    B = x.shape[0]

    idxs, gs = _route(x, wg, bg)
    counts = [len(r) for r in idxs]
    total = sum(counts)

    nc = build_moe(counts)

    # Shared (identical on every core): sorted activations and gates.
    order = np.concatenate(idxs)
    xt_all = np.ascontiguousarray(x[order].T).astype(np.float16)       # (D, total)
    g_all = np.concatenate(gs).astype(np.float16)                      # (total,)
    g_rep = np.ascontiguousarray(np.broadcast_to(g_all, (128, total)))

    in_maps = []
    for c in range(N_CORES):
        # Core c's H-slice [c*512, (c+1)*512) of every expert.
        w1c = np.concatenate([w1[e][:, c * HS:(c + 1) * HS] for e in range(E)], axis=1)
        w2c = np.concatenate([w2[e][c * HS:(c + 1) * HS, :] for e in range(E)], axis=0)
        b1c = np.concatenate([b1[e][c * HS:(c + 1) * HS].reshape(NHS, 128).T
                              for e in range(E)], axis=1)
        in_maps.append({
            "xt": xt_all,
            "w1": w1c.astype(np.float16),
            "w2": w2c.astype(np.float16),
            "b1": np.ascontiguousarray(b1c),
            "g": g_rep,
        })

    kwargs = {}
    if trace:
        kwargs["trace"] = True
        if trace_kwargs:
            kwargs.update(trace_kwargs)
    res = run_bass_kernel_spmd(nc, in_maps, core_ids=list(range(N_CORES)), **kwargs)

    # Sum the 8 cores' H-slice partials, then scatter-add per-expert segments.
    ysum = res.results[0]["yt"].astype(np.float32)
    for c in range(1, N_CORES):
        ysum += res.results[c]["yt"].astype(np.float32)

    out = np.zeros((B, D), np.float32)
    t = 0
    for e in range(E):
        n = counts[e]
        out[idxs[e]] += ysum[:, t:t + n].T + gs[e][:, None] * b2[e][None, :]
        t += n
    return out, res


def kernel(x, wg, bg, w1, b1, w2, b2):
    out, _ = moe_run(x, wg, bg, w1, b1, w2, b2, trace=False)
    return out
